# revision 21
# baseline (speedup 1.0000x reference)
"""Trainium2 Bass kernel for DynamicRoutingAggregator.

Math (per batch b):
  shared = tanh(X @ W + b)                        # [T, C*CD], bf16 in SBUF
  A_0 = 0; for it in 0..2:
    Cw = (it==0) ? mask/16 : softmax_c(shared . A_{it}) * mask
    S  = sum_t Cw[t,c] * shared[t,c,:]            # PE junk-matmul [C, C*CD], diag extracted
    V  = squash(S); A_{it+1} = A_{it} + V
  out = V_3

Key identity: logits_k = shared . (sum_{j<k} V_j) for unmasked tokens, so no
logits storage; masked tokens are killed via the Cw mask multiply.

Schedule: it-0's S junk-matmul is fused chunk-by-chunk into the phase-1
stream (C_0 = mask/16 needs no routing state), so the PE runs one continuous
stream of phase-1 + S matmuls.  it-1/it-2 work for earlier batches is
software-pipelined into later batches' phase-1 blocks at readiness-matched
slots.  The logits d-reduction runs as a pairwise tree on the (otherwise
idle) GpSimd engine; squash uses sqrt(q)/(1+q) with one Sqrt activation.

Sharding: data-parallel over batch, 8 batches per core on 8 cores.
Host prep: X -> bf16, transposed to [D, T] per batch (contraction dim on
partitions); W -> bf16; mask -> [128, T/128] f32 chunk layout.
"""

import sys

sys.path.insert(0, "/opt/trn_rl_repo")

import numpy as np
import ml_dtypes

BF = ml_dtypes.bfloat16

B, T, D = 64, 1024, 512
CAPS, CD = 16, 64
U = CAPS * CD  # 1024
NCORES = 8
NB = B // NCORES  # batches per core

_CACHE = {}


def _build(nb, tch, has_bias, opts=None):
    """Build the Bass program for one core: nb batches, tch token-chunks of 128."""
    import concourse.bacc as bacc
    import concourse.bass as bass
    import concourse.tile as tile
    import concourse.mybir as mybir

    opts = opts or {}
    gp_tree = opts.get("gp_tree", False)
    newton_squash = opts.get("newton_squash", True)
    newton_iters = opts.get("newton_iters", 1)
    fuse_s0 = opts.get("fuse_s0", True)
    f32 = mybir.dt.float32
    bf16 = mybir.dt.bfloat16
    AF = mybir.ActivationFunctionType
    ALU = mybir.AluOpType
    AX = mybir.AxisListType

    tt = tch * 128  # tokens per batch
    dch = D // 128

    nc = bacc.Bacc("TRN2", num_devices=NCORES)
    xt = nc.declare_dram_parameter("xt", [nb, D, tt], bf16, isOutput=False)
    w = nc.declare_dram_parameter("w", [D, U], bf16, isOutput=False)
    mask = nc.declare_dram_parameter("mask", [nb, 128, tch], f32, isOutput=False)
    if has_bias:
        bbc = nc.declare_dram_parameter("bbc", [128, U], f32, isOutput=False)
    out = nc.declare_dram_parameter("out", [nb, U], f32, isOutput=True)

    with tile.TileContext(nc) as tc:
        with (
            tc.tile_pool(name="wp", bufs=1) as wp,
            tc.tile_pool(name="xp", bufs=3) as xp,
            tc.tile_pool(name="mp", bufs=nb) as mp,
            tc.tile_pool(name="shp", bufs=nb) as shp,
            tc.tile_pool(name="c1p", bufs=3) as c1p,
            tc.tile_pool(name="prp", bufs=2) as prp,
            tc.tile_pool(name="ph1p", bufs=2) as ph1p,
            tc.tile_pool(name="ph2p", bufs=2) as ph2p,
            tc.tile_pool(name="ph3p", bufs=2) as ph3p,
            tc.tile_pool(name="lgp", bufs=2) as lgp,
            tc.tile_pool(name="smx", bufs=4) as smx,
            tc.tile_pool(name="cwp", bufs=2) as cwp,
            tc.tile_pool(name="ssp", bufs=2) as ssp,
            tc.tile_pool(name="smp", bufs=12) as smp,
            tc.tile_pool(name="abp", bufs=3) as abp,
            tc.tile_pool(name="adp", bufs=6, space="DRAM") as adp,
            tc.tile_pool(name="mmps", bufs=2, space="PSUM") as mmps,
            tc.tile_pool(name="sps", bufs=3, space="PSUM") as sps,
        ):
            w_sb = wp.tile([128, dch * U], bf16)
            for j in range(dch):
                nc.sync.dma_start(w_sb[:, j * U:(j + 1) * U], w[j * 128:(j + 1) * 128, :])
            if has_bias:
                bb_sb = wp.tile([128, U], f32)
                nc.sync.dma_start(bb_sb[:], bbc[:, :])

            xts, mks = [], []
            for bi in range(nb):
                xt_sb = xp.tile([128, dch * tt], bf16)
                for j in range(dch):
                    nc.sync.dma_start(
                        xt_sb[:, j * tt:(j + 1) * tt], xt[bi, j * 128:(j + 1) * 128, :]
                    )
                mk_sb = mp.tile([128, tch], f32)
                nc.sync.dma_start(mk_sb[:], mask[bi])
                xts.append(xt_sb)
                mks.append(mk_sb)

            shareds = [None] * nb
            cw1s = [None] * nb
            cw2s = {}
            sptiles = {}
            ss_sb = {}
            Ss = {}
            As = [None] * nb
            abcs = {}
            lgs = {}

            def p1_chunk(bi, tci):
                # nh-halved PSUM tiles: [128, 512] = 1 bank each, so sps can
                # triple-buffer within the 8-bank budget.
                shared = shareds[bi]
                for nh in range(2):
                    psh = mmps.tile([128, 512], f32, name="psh")
                    for j in range(dch):
                        lhsT = xts[bi][:, j * tt + tci * 128: j * tt + (tci + 1) * 128]
                        nc.tensor.matmul(
                            psh[:],
                            lhsT=lhsT,
                            rhs=w_sb[:, j * U + nh * 512: j * U + nh * 512 + 512],
                            start=(j == 0),
                            stop=(j == dch - 1),
                        )
                    if has_bias:
                        nc.vector.tensor_add(psh[:], psh[:],
                                             bb_sb[:, nh * 512:(nh + 1) * 512])
                    nc.scalar.activation(
                        shared[:, tci * U + nh * 512: tci * U + nh * 512 + 512],
                        psh[:], AF.Tanh)

            def cw1_make(bi):
                mk_sb = mks[bi]
                cw1 = c1p.tile([128, tch * CAPS], bf16)
                mkv = bass.AP(mk_sb[:].tensor, mk_sb[:].offset,
                              [[tch, 128], [1, tch], [0, CAPS]])
                nc.vector.tensor_scalar_mul(
                    cw1[:].rearrange("p (t c) -> p t c", c=CAPS), mkv, 1.0 / 16.0,
                )
                cw1s[bi] = cw1

            def s_chunk(bi, it, tci):
                """One chunk of the S junk-matmul for (bi, it)."""
                if (bi, it) not in sptiles:
                    sptiles[(bi, it)] = sps.tile([CAPS, U], f32, name="s_ps")
                s_ps = sptiles[(bi, it)]
                cw = cw1s[bi] if it == 0 else cw2s[(bi, it)]
                sh_sl = shareds[bi][:, tci * U:(tci + 1) * U]
                for nh in range(2):
                    nc.tensor.matmul(
                        s_ps[:, nh * 512:(nh + 1) * 512],
                        lhsT=cw[:, tci * CAPS:(tci + 1) * CAPS],
                        rhs=sh_sl[:, nh * 512:(nh + 1) * 512],
                        start=(tci == 0),
                        stop=(tci == tch - 1),
                    )

            def copy_diag(bi, it):
                """PSUM -> SBUF copy then diag-gather DMA: S[c,:] = ssb[c, c*CD:]."""
                s_ps = sptiles.pop((bi, it))
                ssb = ssp.tile([CAPS, U], f32)
                nc.scalar.copy(ssb[:], s_ps[:])
                S = smp.tile([CAPS, CD], f32, tag="S")
                dsrc = bass.AP(ssb[:].tensor, ssb[:].offset,
                               [[U + CD, CAPS], [1, CD]])
                nc.sync.dma_start(S[:], dsrc)
                Ss[(bi, it)] = S

            def back(bi, it):
                """squash(S) -> V; update A; broadcast A (it<2) or emit output."""
                S = Ss.pop((bi, it))
                if not newton_squash:
                    sq = smp.tile([CAPS, CD], f32, tag="sq")
                    q = smp.tile([CAPS, 1], f32, tag="q")
                    # note: tensor_tensor_reduce faults on hardware here; use
                    # separate mul + reduce instead.
                    nc.vector.tensor_mul(sq[:], S[:], S[:])
                    nc.vector.tensor_reduce(q[:], sq[:], axis=AX.X, op=ALU.add)
                    nc.vector.tensor_scalar_add(q[:], q[:], 1e-8)
                    sr = smp.tile([CAPS, 1], f32, tag="sr")
                    nc.scalar.activation(sr[:], q[:], AF.Sqrt)
                    u = smp.tile([CAPS, 1], f32, tag="u")
                    nc.vector.tensor_scalar_add(u[:], q[:], 1.0)
                    nc.vector.reciprocal(u[:], u[:])
                    f = smp.tile([CAPS, 1], f32, tag="f")
                    nc.vector.tensor_mul(f[:], sr[:], u[:])
                    V = smp.tile([CAPS, CD], f32, tag="V")
                    nc.vector.tensor_scalar_mul(V[:], S[:], f[:])
                else:
                    i32 = mybir.dt.int32
                    sq = smp.tile([CAPS, CD], f32, tag="sq")
                    q = smp.tile([CAPS, 1], f32, tag="q")
                    nc.vector.tensor_mul(sq[:], S[:], S[:])
                    nc.vector.tensor_reduce(q[:], sq[:], axis=AX.X, op=ALU.add)
                    y = smp.tile([CAPS, 1], f32, tag="y")
                    t1 = smp.tile([CAPS, 1], f32, tag="t1")
                    nc.vector.tensor_scalar(
                        t1[:].bitcast(i32), q[:].bitcast(i32), 1, -1,
                        op0=ALU.logical_shift_right, op1=ALU.bitwise_xor,
                    )
                    nc.vector.tensor_scalar(
                        y[:].bitcast(i32), t1[:].bitcast(i32), 0x5F3759E0, None,
                        op0=ALU.add,
                    )
                    for _ in range(newton_iters):
                        t2 = smp.tile([CAPS, 1], f32, tag="t2")
                        nc.vector.tensor_mul(t2[:], y[:], y[:])
                        nc.vector.tensor_mul(t2[:], t2[:], q[:])
                        nc.vector.tensor_scalar(t2[:], t2[:], -0.5, 1.5,
                                                op0=ALU.mult, op1=ALU.add)
                        nc.vector.tensor_mul(y[:], y[:], t2[:])
                    f = smp.tile([CAPS, 1], f32, tag="f")
                    u = smp.tile([CAPS, 1], f32, tag="u")
                    nc.vector.tensor_mul(f[:], q[:], y[:])
                    nc.vector.tensor_scalar_add(u[:], q[:], 1.0)
                    nc.vector.reciprocal(u[:], u[:])
                    nc.vector.tensor_mul(f[:], f[:], u[:])
                    V = smp.tile([CAPS, CD], f32, tag="V")
                    nc.vector.tensor_scalar_mul(V[:], S[:], f[:])

                if it == 0:
                    As[bi] = V
                elif it == 1:
                    A2 = smp.tile([CAPS, CD], f32, tag="A2")
                    nc.vector.tensor_add(A2[:], As[bi][:], V[:])
                    As[bi] = A2

                if it < 2:
                    abf = smp.tile([CAPS, CD], bf16, tag="abf")
                    nc.vector.tensor_copy(abf[:], As[bi][:])
                    arow_d = adp.tile([U], bf16)
                    nc.sync.dma_start(arow_d[:], abf[:])
                    abc = abp.tile([128, U], bf16)
                    bsrc = bass.AP(arow_d[:].tensor, arow_d[:].offset,
                                   [[0, 128], [1, U]])
                    nc.sync.dma_start(abc[:], bsrc)
                    abcs[bi] = abc
                else:
                    nc.sync.dma_start(out[bi:bi + 1, :], V[:])

            def front_pre(bi, it):
                """prod = shared*A_bcast; tree-reduce d 64->8; seg-reduce -> lg."""
                shared = shareds[bi]
                abc = abcs[bi]
                prod = prp.tile([128, tch * U], bf16)
                abc_loop = bass.AP(abc[:].tensor, abc[:].offset,
                                   [[U, 128], [0, tch], [1, U]])
                nc.vector.tensor_mul(
                    prod[:].rearrange("p (t u) -> p t u", u=U),
                    shared[:].rearrange("p (t u) -> p t u", u=U),
                    abc_loop,
                )
                eng = nc.gpsimd if gp_tree else nc.vector
                pv = prod[:].rearrange("p (c two d) -> p c two d",
                                       two=2, d=CD // 2)
                ph = ph1p.tile([128, tch * U // 2], bf16)
                eng.tensor_add(
                    ph[:].rearrange("p (c d) -> p c d", d=CD // 2),
                    pv[:, :, 0, :], pv[:, :, 1, :],
                )
                pv2 = ph[:].rearrange("p (c two d) -> p c two d",
                                      two=2, d=CD // 4)
                ph2 = ph2p.tile([128, tch * U // 4], bf16)
                eng.tensor_add(
                    ph2[:].rearrange("p (c d) -> p c d", d=CD // 4),
                    pv2[:, :, 0, :], pv2[:, :, 1, :],
                )
                pv3 = ph2[:].rearrange("p (c two d) -> p c two d",
                                       two=2, d=CD // 8)
                ph3 = ph3p.tile([128, tch * U // 8], bf16)
                eng.tensor_add(
                    ph3[:].rearrange("p (c d) -> p c d", d=CD // 8),
                    pv3[:, :, 0, :], pv3[:, :, 1, :],
                )
                nseg = tch * CAPS
                lg = lgp.tile([128, nseg], f32)
                nc.vector.tensor_reduce(
                    lg[:],
                    ph3[:].rearrange("p (c d) -> p c d", d=CD // 8),
                    axis=AX.X,
                    op=ALU.add,
                )
                lgs[(bi, it)] = lg

            def front_post(bi, it):
                """exp -> softmax denom -> Cw (bf16) for the S matmul."""
                lg = lgs.pop((bi, it))
                mk_sb = mks[bi]
                nseg = tch * CAPS
                eo = smx.tile([128, nseg], f32, tag="eo")
                nc.scalar.activation(eo[:], lg[:], AF.Exp)
                se = smx.tile([128, tch], f32, tag="se")
                nc.vector.tensor_reduce(
                    se[:], eo[:].rearrange("p (t c) -> p t c", c=CAPS),
                    axis=AX.X, op=ALU.add,
                )
                rcm = smx.tile([128, tch], f32, tag="rcm")
                nc.vector.reciprocal(rcm[:], se[:])
                nc.vector.tensor_mul(rcm[:], rcm[:], mk_sb[:])
                cw = cwp.tile([128, tch * CAPS], bf16)
                rcm_loop = bass.AP(rcm[:].tensor, rcm[:].offset,
                                   [[tch, 128], [1, tch], [0, CAPS]])
                nc.vector.tensor_mul(
                    cw[:].rearrange("p (t c) -> p t c", c=CAPS),
                    eo[:].rearrange("p (t c) -> p t c", c=CAPS),
                    rcm_loop,
                )
                cw2s[(bi, it)] = cw

            def smm_mms(bi, it):
                for tci in range(tch):
                    s_chunk(bi, it, tci)

            # ---- software-pipelined emission -------------------------------
            # Block B = batch B's phase-1 (+fused it-0 S chunks).  Compressed
            # lifecycle: batch b runs it-1 in block b+1 and it-2 in block b+2.
            # Exps (front_post) are emitted before the PSUM copies so the
            # scalar queue never head-of-line-blocks the cw chain; all copies
            # group at block end (copy0 first -- its matmul stops earliest).
            nblk = nb + 3
            for Bb in range(nblk):
                in_p = Bb < nb

                def slot(k):
                    if k == 0:
                        if 0 <= Bb - 1 < nb:
                            back(Bb - 1, 0)
                        if 0 <= Bb - 2 < nb:
                            back(Bb - 2, 1)
                        if 0 <= Bb - 3 < nb:
                            back(Bb - 3, 2)
                    elif k == 2:
                        if 0 <= Bb - 1 < nb:
                            front_pre(Bb - 1, 1)
                        if 0 <= Bb - 2 < nb:
                            front_pre(Bb - 2, 2)

                def endslot():
                    if (Bb - 1, 1) in lgs:
                        front_post(Bb - 1, 1)
                    if (Bb - 2, 2) in lgs:
                        front_post(Bb - 2, 2)
                    if 0 <= Bb - 1 < nb:
                        smm_mms(Bb - 1, 1)
                    if 0 <= Bb - 2 < nb:
                        smm_mms(Bb - 2, 2)
                    if in_p:
                        copy_diag(Bb, 0)
                    if 0 <= Bb - 1 < nb:
                        copy_diag(Bb - 1, 1)
                    if 0 <= Bb - 2 < nb:
                        copy_diag(Bb - 2, 2)

                if in_p:
                    sh_new = shp.tile([128, tch * U], bf16, tag="shared")
                    shareds[Bb] = sh_new
                    cw1_make(Bb)
                    for tci in range(tch):
                        p1_chunk(Bb, tci)
                        if fuse_s0 and tci >= 1:
                            s_chunk(Bb, 0, tci - 1)
                        if tci < 4:
                            slot(tci)
                    if fuse_s0:
                        s_chunk(Bb, 0, tch - 1)
                    else:
                        for tci in range(tch):
                            s_chunk(Bb, 0, tci)
                    endslot()
                else:
                    slot(0)
                    slot(2)
                    endslot()

    nc.compile()
    return nc


def _get_nc(nb, tch, has_bias, opts=None):
    key = (nb, tch, has_bias, tuple(sorted((opts or {}).items())))
    if key not in _CACHE:
        _CACHE[key] = _build(nb, tch, has_bias, opts)
    return _CACHE[key]


def _prep_core_inputs(X, mask, W, b, nb, tch):
    """Host-side prep for one core's slice. X [nb,tt,D] f32 -> dict of arrays."""
    tt = tch * 128
    assert X.shape[1] == tt and mask.shape[1] == tt
    xt = np.ascontiguousarray(
        X.astype(BF).transpose(0, 2, 1)
    )  # [nb, D, tt] bf16
    mk = np.ascontiguousarray(
        mask.astype(np.float32).reshape(nb, tch, 128).transpose(0, 2, 1)
    )  # [nb, 128, tch]
    d = {"xt": xt, "mask": mk}
    return d


COMPACT_TCH = 5  # 640 token slots; batches with more surviving tokens fall back


def _compact(X, mask, tt):
    """Keep only unmasked tokens, zero-pad to tt. Returns (Xc, maskc) or None."""
    Bn = X.shape[0]
    Xc = np.zeros((Bn, tt, X.shape[2]), np.float32)
    mc = np.zeros((Bn, tt), np.int32)
    for i in range(Bn):
        idx = np.flatnonzero(mask[i])
        if len(idx) > tt:
            return None
        Xc[i, :len(idx)] = X[i, idx]
        mc[i, :len(idx)] = 1
    return Xc, mc


def kernel(input_tensors, mask, W, b):
    input_tensors = np.asarray(input_tensors, dtype=np.float32)
    mask = np.asarray(mask)
    W = np.asarray(W, dtype=np.float32)
    b = np.asarray(b, dtype=np.float32)

    has_bias = bool(np.any(b != 0.0))
    comp = _compact(input_tensors, mask, COMPACT_TCH * 128)
    if comp is not None:
        input_tensors, mask = comp
        tch = COMPACT_TCH
    else:
        tch = T // 128
    import os
    opts = {}
    if os.environ.get("K_GP_TREE", "0") == "1":
        opts["gp_tree"] = True
    if os.environ.get("K_NEWTON", "1") == "0":
        opts["newton_squash"] = False
    if os.environ.get("K_NOFUSE", "0") == "1":
        opts["fuse_s0"] = False
    nc = _get_nc(NB, tch, has_bias, opts)

    wb = np.ascontiguousarray(W.astype(BF))  # [D, U] bf16
    in_maps = []
    for core in range(NCORES):
        sl = slice(core * NB, (core + 1) * NB)
        d = _prep_core_inputs(input_tensors[sl], mask[sl], W, b, NB, tch)
        d["w"] = wb
        if has_bias:
            d["bbc"] = np.broadcast_to(b.astype(np.float32), (128, U)).copy()
        in_maps.append(d)

    from concourse.bass_utils import run_bass_kernel_spmd

    res = run_bass_kernel_spmd(nc, in_maps, list(range(NCORES)))
    out = np.concatenate([np.asarray(res.results[i]["out"]) for i in range(NCORES)], 0)
    return out.astype(np.float32)


if __name__ == "__main__":
    rng = np.random.default_rng(0)
    X = rng.standard_normal((B, T, D), dtype=np.float32)
    mk = rng.integers(0, 2, (B, T)).astype(np.int32)
    Wm = (rng.standard_normal((D, U), dtype=np.float32) / np.sqrt(D)).astype(np.float32)
    bv = np.zeros((U,), np.float32)
    o = kernel(X, mk, Wm, bv)
    print("out", o.shape, o.dtype, np.abs(o).max())


# revision 22
# speedup vs baseline: 1.1420x; 1.1420x over previous
"""Trainium2 Bass kernel for DynamicRoutingAggregator.

Math (per batch b):
  shared = tanh(X @ W + b)                        # [T, C*CD], bf16 in SBUF
  A_0 = 0; for it in 0..2:
    Cw = (it==0) ? mask/16 : softmax_c(shared . A_{it}) * mask
    S  = sum_t Cw[t,c] * shared[t,c,:]            # PE junk-matmul [C, C*CD], diag extracted
    V  = squash(S); A_{it+1} = A_{it} + V
  out = V_3

Key identity: logits_k = shared . (sum_{j<k} V_j) for unmasked tokens, so no
logits storage; masked tokens are killed via the Cw mask multiply.

Sharding: data-parallel over batch, 8 batches per core on 8 cores.
Host prep: X -> bf16, transposed to [D, T] per batch (contraction dim on
partitions); W -> bf16; mask -> [128, T/128] f32 chunk layout.

Squash rsqrt: bit-hack seed + one Newton step (seed is within ~3.5%, one
step lands ~0.2% which is far inside the output tolerance); the shift/xor
pair is fused into a single two-op tensor_scalar and the +1e-8 bias is
dropped (q == 0 still yields V == 0 exactly via f = q*y = 0).
"""

import sys

sys.path.insert(0, "/opt/trn_rl_repo")

import numpy as np
import ml_dtypes

BF = ml_dtypes.bfloat16

B, T, D = 64, 1024, 512
CAPS, CD = 16, 64
U = CAPS * CD  # 1024
NCORES = 8
NB = B // NCORES  # batches per core

_CACHE = {}


def _build(nb, tch, has_bias, opts=None):
    """Build the Bass program for one core: nb batches, tch token-chunks of 128."""
    import concourse.bacc as bacc
    import concourse.bass as bass
    import concourse.tile as tile
    import concourse.mybir as mybir

    opts = opts or {}
    f32 = mybir.dt.float32
    bf16 = mybir.dt.bfloat16
    i32 = mybir.dt.int32
    AF = mybir.ActivationFunctionType
    ALU = mybir.AluOpType
    AX = mybir.AxisListType

    tt = tch * 128  # tokens per batch
    dch = D // 128

    nc = bacc.Bacc("TRN2", num_devices=NCORES)
    xt = nc.declare_dram_parameter("xt", [nb, D, tt], bf16, isOutput=False)
    w = nc.declare_dram_parameter("w", [D, U], bf16, isOutput=False)
    mask = nc.declare_dram_parameter("mask", [nb, 128, tch], f32, isOutput=False)
    if has_bias:
        bbc = nc.declare_dram_parameter("bbc", [128, U], f32, isOutput=False)
    out = nc.declare_dram_parameter("out", [nb, U], f32, isOutput=True)

    with tile.TileContext(nc) as tc:
        with (
            tc.tile_pool(name="wp", bufs=1) as wp,
            tc.tile_pool(name="xp", bufs=3) as xp,
            tc.tile_pool(name="mp", bufs=8) as mp,
            tc.tile_pool(name="shp", bufs=8) as shp,
            tc.tile_pool(name="cwp", bufs=10) as cwp,
            tc.tile_pool(name="prp", bufs=3) as prp,
            tc.tile_pool(name="lgp", bufs=10) as lgp,
            tc.tile_pool(name="smp", bufs=10) as smp,
            tc.tile_pool(name="abp", bufs=6) as abp,
            tc.tile_pool(name="ssp", bufs=2) as ssp,
            tc.tile_pool(name="adp", bufs=6, space="DRAM") as adp,
            tc.tile_pool(name="mmps", bufs=2, space="PSUM") as mmps,
            tc.tile_pool(name="sps", bufs=2, space="PSUM") as sps,
        ):
            w_sb = wp.tile([128, dch * U], bf16)
            for j in range(dch):
                nc.sync.dma_start(w_sb[:, j * U:(j + 1) * U], w[j * 128:(j + 1) * 128, :])
            if has_bias:
                bb_sb = wp.tile([128, U], f32)
                nc.sync.dma_start(bb_sb[:], bbc[:, :])

            xts, mks, shareds = [], [], []
            for bi in range(nb):
                xt_sb = xp.tile([128, dch * tt], bf16)
                for j in range(dch):
                    nc.sync.dma_start(
                        xt_sb[:, j * tt:(j + 1) * tt], xt[bi, j * 128:(j + 1) * 128, :]
                    )
                mk_sb = mp.tile([128, tch], f32)
                nc.sync.dma_start(mk_sb[:], mask[bi])
                xts.append(xt_sb)
                mks.append(mk_sb)
                shareds.append(None)

            As = [None] * nb
            abcs = [None] * nb

            def phase1_chunk(bi, tci):
                shared = shareds[bi]
                ps = mmps.tile([128, U], f32)
                for j in range(dch):
                    lhsT = xts[bi][:, j * tt + tci * 128: j * tt + (tci + 1) * 128]
                    for nh in range(2):
                        nc.tensor.matmul(
                            ps[:, nh * 512:(nh + 1) * 512],
                            lhsT=lhsT,
                            rhs=w_sb[:, j * U + nh * 512: j * U + nh * 512 + 512],
                            start=(j == 0),
                            stop=(j == dch - 1),
                        )
                if has_bias:
                    nc.vector.tensor_add(ps[:], ps[:], bb_sb[:])
                nc.scalar.activation(shared[:, tci * U:(tci + 1) * U], ps[:], AF.Tanh)

            Ss = [None] * nb

            def iter_front(bi, it):
                shared = shareds[bi]
                mk_sb = mks[bi]
                abc = abcs[bi]
                s_ps = sps.tile([CAPS, U], f32)
                if it == 0:
                    cw1_all = cwp.tile([128, tch * CAPS], bf16, tag="cw1")
                    mkv = bass.AP(mk_sb[:].tensor, mk_sb[:].offset,
                                  [[tch, 128], [1, tch], [0, CAPS]])
                    nc.vector.tensor_scalar_mul(
                        cw1_all[:].rearrange("p (t c) -> p t c", c=CAPS), mkv,
                        1.0 / 16.0,
                    )
                    cws = [cw1_all[:, tci * CAPS:(tci + 1) * CAPS]
                           for tci in range(tch)]
                else:
                    # whole-batch logits: prod = shared * A_bcast (A looped via
                    # stride-0 mid-dim), pairwise tree-add, segmented reduce
                    prod = prp.tile([128, tch * U], bf16)
                    abc_loop = bass.AP(abc[:].tensor, abc[:].offset,
                                       [[U, 128], [0, tch], [1, U]])
                    nc.vector.tensor_mul(
                        prod[:].rearrange("p (t u) -> p t u", u=U),
                        shared[:].rearrange("p (t u) -> p t u", u=U),
                        abc_loop,
                    )
                    pv = prod[:].rearrange("p (c two d) -> p c two d",
                                           two=2, d=CD // 2)
                    ph = prp.tile([128, tch * U // 2], bf16, tag="ph")
                    nc.vector.tensor_add(
                        ph[:].rearrange("p (c d) -> p c d", d=CD // 2),
                        pv[:, :, 0, :], pv[:, :, 1, :],
                    )
                    pv2 = ph[:].rearrange("p (c two d) -> p c two d",
                                          two=2, d=CD // 4)
                    ph2 = prp.tile([128, tch * U // 4], bf16, tag="ph2")
                    nc.vector.tensor_add(
                        ph2[:].rearrange("p (c d) -> p c d", d=CD // 4),
                        pv2[:, :, 0, :], pv2[:, :, 1, :],
                    )
                    pv3 = ph2[:].rearrange("p (c two d) -> p c two d",
                                           two=2, d=CD // 8)
                    ph3 = prp.tile([128, tch * U // 8], bf16, tag="ph3")
                    nc.vector.tensor_add(
                        ph3[:].rearrange("p (c d) -> p c d", d=CD // 8),
                        pv3[:, :, 0, :], pv3[:, :, 1, :],
                    )
                    nseg = tch * CAPS
                    lg = lgp.tile([128, nseg], f32)
                    nc.vector.tensor_reduce(
                        lg[:],
                        ph3[:].rearrange("p (c d) -> p c d", d=CD // 8),
                        axis=AX.X,
                        op=ALU.add,
                    )
                    eo = lgp.tile([128, nseg], f32)
                    nc.scalar.activation(eo[:], lg[:], AF.Exp)
                    se = lgp.tile([128, tch], f32)
                    nc.vector.tensor_reduce(
                        se[:], eo[:].rearrange("p (t c) -> p t c", c=CAPS),
                        axis=AX.X, op=ALU.add,
                    )
                    rc = lgp.tile([128, tch], f32)
                    nc.vector.reciprocal(rc[:], se[:])
                    rcm = lgp.tile([128, tch], f32, tag="rcm")
                    nc.vector.tensor_mul(rcm[:], rc[:], mk_sb[:])
                    cw_all = cwp.tile([128, tch * CAPS], bf16, tag="cw1")
                    rcm_loop = bass.AP(rcm[:].tensor, rcm[:].offset,
                                       [[tch, 128], [1, tch], [0, CAPS]])
                    nc.vector.tensor_mul(
                        cw_all[:].rearrange("p (t c) -> p t c", c=CAPS),
                        eo[:].rearrange("p (t c) -> p t c", c=CAPS),
                        rcm_loop,
                    )
                    cws = [cw_all[:, tci * CAPS:(tci + 1) * CAPS]
                           for tci in range(tch)]
                for tci in range(tch):
                    sh_sl = shared[:, tci * U:(tci + 1) * U]
                    for nh in range(2):
                        nc.tensor.matmul(
                            s_ps[:, nh * 512:(nh + 1) * 512],
                            lhsT=cws[tci],
                            rhs=sh_sl[:, nh * 512:(nh + 1) * 512],
                            start=(tci == 0),
                            stop=(tci == tch - 1),
                        )

                # diag extract: S[c, :] = s_ps[c, c*CD:(c+1)*CD]
                S = smp.tile([CAPS, CD], f32)
                ssb = ssp.tile([CAPS, U], f32)
                nc.scalar.copy(ssb[:], s_ps[:])
                dsrc = bass.AP(ssb[:].tensor, ssb[:].offset,
                               [[U + CD, CAPS], [1, CD]])
                nc.sync.dma_start(S[:], dsrc)
                Ss[bi] = S

            def iter_back(bi, it):
                S = Ss[bi]
                # squash: V = (q/(1+q)) * S / sqrt(q), q = |S|^2
                sq = smp.tile([CAPS, CD], f32)
                q = smp.tile([CAPS, 1], f32)
                nc.vector.tensor_mul(sq[:], S[:], S[:])
                nc.vector.tensor_reduce(q[:], sq[:], axis=AX.X, op=ALU.add)
                y = smp.tile([CAPS, 1], f32)
                t1 = smp.tile([CAPS, 1], f32)
                nc.vector.tensor_scalar(
                    t1[:].bitcast(i32), q[:].bitcast(i32), 1, -1,
                    op0=ALU.logical_shift_right, op1=ALU.bitwise_xor,
                )
                nc.vector.tensor_scalar(
                    y[:].bitcast(i32), t1[:].bitcast(i32), 0x5F3759E0, None,
                    op0=ALU.add,
                )
                t2 = smp.tile([CAPS, 1], f32)
                nc.vector.tensor_mul(t2[:], y[:], y[:])
                nc.vector.tensor_mul(t2[:], t2[:], q[:])
                nc.vector.tensor_scalar(t2[:], t2[:], -0.5, 1.5,
                                        op0=ALU.mult, op1=ALU.add)
                nc.vector.tensor_mul(y[:], y[:], t2[:])
                f = smp.tile([CAPS, 1], f32)
                u = smp.tile([CAPS, 1], f32)
                nc.vector.tensor_mul(f[:], q[:], y[:])
                nc.vector.tensor_scalar_add(u[:], q[:], 1.0)
                nc.vector.reciprocal(u[:], u[:])
                nc.vector.tensor_mul(f[:], f[:], u[:])
                V = smp.tile([CAPS, CD], f32)
                nc.vector.tensor_scalar_mul(V[:], S[:], f[:])

                if it == 0:
                    As[bi] = V
                elif it == 1:
                    A2 = smp.tile([CAPS, CD], f32)
                    nc.vector.tensor_add(A2[:], As[bi][:], V[:])
                    As[bi] = A2

                if it < 2:
                    abf = smp.tile([CAPS, CD], bf16)
                    nc.vector.tensor_copy(abf[:], As[bi][:])
                    arow_d = adp.tile([U], bf16)
                    nc.sync.dma_start(arow_d[:], abf[:])
                    abc = abp.tile([128, U], bf16)
                    bsrc = bass.AP(arow_d[:].tensor, arow_d[:].offset,
                                   [[0, 128], [1, U]])
                    nc.sync.dma_start(abc[:], bsrc)
                    abcs[bi] = abc
                else:
                    nc.sync.dma_start(out[bi:bi + 1, :], V[:])

            # skewed wavefront: cadence-2 in steady state (a full wave
            # between a back's A-broadcast DMAs and the consuming front),
            # cadence-1 for edge batches where stalls land in idle waves

            def cad(b):
                return 2

            fronts, backs = {}, {}
            maxw = 0
            for b in range(nb):
                for it in range(3):
                    wf = b + 1 + cad(b) * it
                    fronts.setdefault(wf, []).append((b, it))
                    backs.setdefault(wf + 1, []).append((b, it))
                    maxw = max(maxw, wf + 1)
            for w in range(maxw + 1):
                # interleave phase-1 chunks between iteration blocks so PE
                # never idles long enough for HAM to re-throttle
                chunks = list(range(tch)) if w < nb else []
                if w < nb:
                    sh_new = shp.tile([128, tch * U], bf16, tag="shared")
                    shareds[w] = sh_new
                    phase1_chunk(w, chunks.pop(0))
                    phase1_chunk(w, chunks.pop(0))
                for b, it in sorted(backs.get(w, []), key=lambda x: x[1]):
                    iter_back(b, it)
                for b, it in sorted(fronts.get(w, []), key=lambda x: x[1]):
                    iter_front(b, it)
                    if chunks:
                        phase1_chunk(w, chunks.pop(0))
                while chunks:
                    phase1_chunk(w, chunks.pop(0))

    nc.compile()
    return nc


def _get_nc(nb, tch, has_bias, opts=None):
    key = (nb, tch, has_bias, tuple(sorted((opts or {}).items())))
    if key not in _CACHE:
        _CACHE[key] = _build(nb, tch, has_bias, opts)
    return _CACHE[key]


def _prep_core_inputs(X, mask, W, b, nb, tch):
    """Host-side prep for one core's slice. X [nb,tt,D] f32 -> dict of arrays."""
    tt = tch * 128
    assert X.shape[1] == tt and mask.shape[1] == tt
    xt = np.ascontiguousarray(
        X.astype(BF).transpose(0, 2, 1)
    )  # [nb, D, tt] bf16
    mk = np.ascontiguousarray(
        mask.astype(np.float32).reshape(nb, tch, 128).transpose(0, 2, 1)
    )  # [nb, 128, tch]
    d = {"xt": xt, "mask": mk}
    return d


COMPACT_TCH = 5  # 640 token slots; batches with more surviving tokens fall back


def _compact(X, mask, tt):
    """Keep only unmasked tokens, zero-pad to tt. Returns (Xc, maskc) or None."""
    Bn = X.shape[0]
    Xc = np.zeros((Bn, tt, X.shape[2]), np.float32)
    mc = np.zeros((Bn, tt), np.int32)
    for i in range(Bn):
        idx = np.flatnonzero(mask[i])
        if len(idx) > tt:
            return None
        Xc[i, :len(idx)] = X[i, idx]
        mc[i, :len(idx)] = 1
    return Xc, mc


def kernel(input_tensors, mask, W, b):
    input_tensors = np.asarray(input_tensors, dtype=np.float32)
    mask = np.asarray(mask)
    W = np.asarray(W, dtype=np.float32)
    b = np.asarray(b, dtype=np.float32)

    has_bias = bool(np.any(b != 0.0))
    comp = _compact(input_tensors, mask, COMPACT_TCH * 128)
    if comp is not None:
        input_tensors, mask = comp
        tch = COMPACT_TCH
    else:
        tch = T // 128
    nc = _get_nc(NB, tch, has_bias)

    wb = np.ascontiguousarray(W.astype(BF))  # [D, U] bf16
    in_maps = []
    for core in range(NCORES):
        sl = slice(core * NB, (core + 1) * NB)
        d = _prep_core_inputs(input_tensors[sl], mask[sl], W, b, NB, tch)
        d["w"] = wb
        if has_bias:
            d["bbc"] = np.broadcast_to(b.astype(np.float32), (128, U)).copy()
        in_maps.append(d)

    from concourse.bass_utils import run_bass_kernel_spmd

    res = run_bass_kernel_spmd(nc, in_maps, list(range(NCORES)))
    out = np.concatenate([np.asarray(res.results[i]["out"]) for i in range(NCORES)], 0)
    return out.astype(np.float32)


if __name__ == "__main__":
    rng = np.random.default_rng(0)
    X = rng.standard_normal((B, T, D), dtype=np.float32)
    mk = rng.integers(0, 2, (B, T)).astype(np.int32)
    Wm = (rng.standard_normal((D, U), dtype=np.float32) / np.sqrt(D)).astype(np.float32)
    bv = np.zeros((U,), np.float32)
    o = kernel(X, mk, Wm, bv)
    print("out", o.shape, o.dtype, np.abs(o).max())
